# revision 1
# baseline (speedup 1.0000x reference)
"""Bahdanau additive attention on 8 TRN2 NeuronCores (Bass/Tile).

Reference computation (B=4, T=512, S=512, D=256, IN=512):
    wq[b,t,d]   = sum_i x[b,t,i]   * Wq[d,i]
    uh[b,s,d]   = sum_m mems[b,s,m]* Wc[d,m] + bc[d]
    align[b,t,s]= sum_d v[d] * tanh(wq[b,t,d] + uh[b,s,d])     (masked s>=L_b -> -inf)
    av          = softmax_s(align)
    c[b,t,m]    = sum_s av[b,t,s] * mems[b,s,m]
    attn[b,t,o] = sum_k [c|x][b,t,k] * Wout[o,k] + bout[o]
    returns (attn, av)

Sharding: every core takes 64 t-rows from EACH batch (512 rows / 8 cores),
arranged as 2 slots x 128 rows; slot j holds segments (batch 2j, rows 0-63)
and (batch 2j+1, rows 64-127).  The per-batch attention width is truncated
at compile time to S_eff_b = ceil(L_b/2)*2 columns, so masked columns
are (almost) never computed: the tanh payload per core is
64*2*sum_b(S_eff_b) lanes instead of the dense 64*2*4*512.  The kernel is
compiled per S_eff tuple (from the actual mem_masks at call time) and the
program is identical on all cores, so SPMD still holds for any masks.

Per (t, d-half): DVE broadcast-add z = uh_seg + wq[t] (bf16, 4x mode), ACT
tanh batched over many slices (one big ACTIVATE), PE reduces over the
d-partition dim with a 32-column one-hot v weight into the PSUM row for t.
The mask is two additive rank-1 matmuls (segment indicator x 0/-30 row) on
the align PSUM -- unwritten tail columns of short segments are SET by the
first mask matmul via PSUM has_written semantics; softmax sum is fused into
the Exp via accum_out; bout is a rank-1 matmul.  All matmul inputs bf16,
accumulation fp32.  DRAM inputs are pre-laid host-side in the exact SBUF
[128, ...] layout so every DMA is contiguous.
"""
import numpy as np
import ml_dtypes
from contextlib import ExitStack

import concourse.bass as bass
import concourse.bacc as bacc
import concourse.mybir as mybir
import concourse.tile as tile
from concourse.bass_utils import run_bass_kernel_spmd

F32 = mybir.dt.float32
BF16 = mybir.dt.bfloat16
TANH = mybir.ActivationFunctionType.Tanh
EXP = mybir.ActivationFunctionType.Exp
BF = ml_dtypes.bfloat16

B, T, S, D, IN = 4, 512, 512, 256, 512
NC = 8           # cores
NJ = 2           # t-tile slots per core
TT = 128         # t rows per slot
SEG = 64         # rows per segment (batch) within a slot

# ACT batch sizes (t's per ACTIVATE).  Small groups at the stream edges: the
# first tanh starts sooner and the final PE v-matmul chase is short.
GROUPS_HEAD = [2, 2, 4] + [8] * 15
GROUPS_TAIL = [8] * 15 + [4, 2, 2]
assert sum(GROUPS_HEAD) == TT and sum(GROUPS_TAIL) == TT
GMAX = max(GROUPS_HEAD)

_BUILT = {}
LAST_RESULT = None


def _build(seffs):
    """seffs: tuple of 4 per-batch attention widths (multiples of 128)."""
    nc = bacc.Bacc("TRN2", target_bir_lowering=False, debug=False,
                   enable_asserts=False, num_devices=NC)

    xT_d = nc.dram_tensor("xT", [NJ, 4, 128, TT], BF16, kind="ExternalInput")
    memsT_d = nc.dram_tensor("memsT", [NJ, 2, 2, 128, S], BF16,
                             kind="ExternalInput")
    memsL_d = nc.dram_tensor("memsL", [NJ, 2, 128, 4, D], BF16,
                             kind="ExternalInput")
    maskseg_d = nc.dram_tensor("maskseg", [NJ, 2, 1, S], BF16,
                               kind="ExternalInput")
    indic_d = nc.dram_tensor("indic", [2, 1, 128], BF16, kind="ExternalInput")
    ones_d = nc.dram_tensor("ones1", [1, 128], BF16, kind="ExternalInput")
    boutw_d = nc.dram_tensor("boutw", [1, IN], BF16, kind="ExternalInput")
    WqT_d = nc.dram_tensor("WqT", [4, 128, D], BF16, kind="ExternalInput")
    WcT_d = nc.dram_tensor("WcT", [2, 128, D], BF16, kind="ExternalInput")
    vcols_d = nc.dram_tensor("vcols", [128, 2, 32, 32], BF16, kind="ExternalInput")
    WoCT_d = nc.dram_tensor("WoCT", [128, 2, IN], BF16, kind="ExternalInput")
    WoXT_d = nc.dram_tensor("WoXT", [128, 4, IN], BF16, kind="ExternalInput")
    ident_d = nc.dram_tensor("ident", [128, 128], BF16, kind="ExternalInput")
    bc_d = nc.dram_tensor("bc2", [128, 2], F32, kind="ExternalInput")

    attn_d = nc.dram_tensor("attn_outT", [NJ, 128, 4, TT], F32, kind="ExternalOutput")
    align_d = nc.dram_tensor("align_out", [NJ, 128, S], F32, kind="ExternalOutput")

    def seff(j, k):
        return seffs[2 * j + k]

    with tile.TileContext(nc) as tc, ExitStack() as ctx:
        const = ctx.enter_context(tc.tile_pool(name="const", bufs=1))
        pin = ctx.enter_context(tc.tile_pool(name="pin", bufs=2))
        pseg = ctx.enter_context(tc.tile_pool(name="pseg", bufs=1))
        pmid = ctx.enter_context(tc.tile_pool(name="pmid", bufs=4))
        pz = ctx.enter_context(tc.tile_pool(name="pz", bufs=4))
        pth = ctx.enter_context(tc.tile_pool(name="pth", bufs=4))
        pep = ctx.enter_context(tc.tile_pool(name="pep", bufs=NJ))
        psA = ctx.enter_context(tc.tile_pool(name="psA", bufs=NJ, space="PSUM"))
        psU = ctx.enter_context(tc.tile_pool(name="psU", bufs=2, space="PSUM"))
        psWC = ctx.enter_context(tc.tile_pool(name="psWC", bufs=1, space="PSUM"))
        psT = ctx.enter_context(tc.tile_pool(name="psT", bufs=2, space="PSUM"))
        psO = ctx.enter_context(tc.tile_pool(name="psO", bufs=1, space="PSUM"))

        def load(pool, shape, dt, src, tag, engine=None):
            t = pool.tile(shape, dt, tag=tag)
            (engine or nc.sync).dma_start(t[...], src)
            return t

        # startup-critical: wq path (xT0+WqT) and slot0-segA uh path
        # (memsT[0,0]+WcT), balanced across the two DMA queues, chunked
        bc2 = load(const, [128, 2], F32, bc_d.ap(), "bc2")
        xT0c, wqTc, wcTc = [None] * 4, [None] * 4, [None] * 2
        wcTc[0] = load(const, [128, D], BF16, WcT_d.ap()[0], "wcTc0", nc.gpsimd)
        mT00 = [load(pseg, [128, seff(0, 0)], BF16,
                     memsT_d.ap()[0][0][mc][:, :seff(0, 0)], f"mT00c{mc}",
                     nc.gpsimd if mc == 0 else None)
                for mc in range(1)]
        mT00.append(None)
        xT0c[0] = load(pin, [128, TT], BF16, xT_d.ap()[0][0], "xT0c0")
        wqTc[0] = load(const, [128, D], BF16, WqT_d.ap()[0], "wqTc0")
        mT00[1] = load(pseg, [128, seff(0, 0)], BF16,
                       memsT_d.ap()[0][0][1][:, :seff(0, 0)], "mT00c1")
        wcTc[1] = load(const, [128, D], BF16, WcT_d.ap()[1], "wcTc1")
        xT0c[1] = load(pin, [128, TT], BF16, xT_d.ap()[0][1], "xT0c1")
        wqTc[1] = load(const, [128, D], BF16, WqT_d.ap()[1], "wqTc1")
        for ic in range(2, 4):
            xT0c[ic] = load(pin, [128, TT], BF16, xT_d.ap()[0][ic], f"xT0c{ic}",
                            nc.gpsimd)
            wqTc[ic] = load(const, [128, D], BF16, WqT_d.ap()[ic], f"wqTc{ic}",
                            nc.gpsimd)
        vcols = load(const, [128, 2, 32, 32], BF16, vcols_d.ap(), "vcols")
        masksegs = [[load(pin, [1, S], BF16, maskseg_d.ap()[j][k], f"maskseg{k}")
                     for k in range(2)] for j in range(NJ)]
        indics = [load(const, [1, 128], BF16, indic_d.ap()[k], f"indic{k}")
                  for k in range(2)]
        ones1 = load(const, [1, 128], BF16, ones_d.ap(), "ones1")
        boutw = load(const, [1, IN], BF16, boutw_d.ap(), "boutw")

        # remaining memsT segments + everything else off the critical path
        mTs = {(0, 0): mT00}
        for (j, k) in [(0, 1), (1, 0), (1, 1)]:
            mTs[(j, k)] = [load(pseg, [128, seff(j, k)], BF16,
                                memsT_d.ap()[j][k][mc][:, :seff(j, k)],
                                f"mT{j}{k}c{mc}", nc.gpsimd)
                           for mc in range(2)]
        xT1c = [load(pin, [128, TT], BF16, xT_d.ap()[1][ic], f"xT1c{ic}",
                     nc.gpsimd) for ic in range(4)]
        woCT = load(const, [128, 2, IN], BF16, WoCT_d.ap(), "woCT", nc.gpsimd)
        woXT = load(const, [128, 4, IN], BF16, WoXT_d.ap(), "woXT", nc.gpsimd)
        ident = load(const, [128, 128], BF16, ident_d.ap(), "ident", nc.gpsimd)
        memsLs = {}
        for j in range(NJ):
            for k in range(2):
                nch = (seff(j, k) + 127) // 128
                memsLs[(j, k)] = load(pseg, [128, nch, D], BF16,
                                      memsL_d.ap()[j][k][:, :nch, :],
                                      f"memsL{j}{k}", nc.gpsimd)
        xTc = [xT0c, xT1c]

        def phase1_wq(j):
            wq_ps = psWC.tile([128, 2 * TT], F32, tag="wqc_ps")
            for h in range(2):
                for ic in range(4):
                    nc.tensor.matmul(wq_ps[:, h * TT:(h + 1) * TT],
                                     wqTc[ic][:, h * 128:(h + 1) * 128],
                                     xTc[j][ic][...],
                                     start=(ic == 0), stop=(ic == 3))
            wq_sb = pmid.tile([128, 2, TT], F32, tag="wq_sb")
            nc.vector.tensor_copy(wq_sb[...], wq_ps[...])
            return wq_sb

        def phase1_uh(j, k):
            sk = seff(j, k)
            uh_sb = pmid.tile([128, 2, sk], BF16, tag="uh_sb",
                              padded_shape=[128, 2, S], name=f"uh_sb{j}{k}")
            for h in range(2):
                uh_ps = psU.tile([128, sk], F32, tag="uh_ps",
                                 padded_shape=[128, S], name=f"uh_ps{j}{k}{h}")
                for mc in range(2):
                    nc.tensor.matmul(uh_ps[...],
                                     wcTc[mc][:, h * 128:(h + 1) * 128],
                                     mTs[(j, k)][mc][...],
                                     start=(mc == 0), stop=(mc == 1))
                nc.vector.tensor_scalar_add(uh_sb[:, h, :], uh_ps[...],
                                            bc2[:, h:h + 1])
            return uh_sb

        wq_sbs = [phase1_wq(0), None]
        uh_sbs = {(0, 0): phase1_uh(0, 0)}

        def build_groups(j):
            # edge groups stay small (fast first tanh, short final chase);
            # middle groups packed to the z-tile budget, capped at 12 t's
            # so the DVE z-feed never lags one jumbo ACTIVATE
            head = [2, 2, 4] if j == 0 else []
            tail = [4, 2, 2] if j == NJ - 1 else []
            budget = GMAX * 2 * S
            out, t = [], 0
            for g in head:
                out.append((t, g)); t += g
            t_end = TT - sum(tail)
            cur, fd = 0, 0
            while t + cur < t_end:
                w = 2 * seff(j, (t + cur) // SEG)
                if cur > 0 and (fd + w > budget or cur >= 12):
                    out.append((t, cur)); t += cur; cur, fd = 0, 0
                cur += 1; fd += w
            if cur:
                out.append((t, cur)); t += cur
            for g in tail:
                out.append((t, g)); t += g
            assert t == TT
            return out
        groups_of = [build_groups(j) for j in range(NJ)]
        max_fd = max(sum(2 * seff(j, (t0 + i) // SEG) for i in range(gsz))
                     for j in range(NJ) for (t0, gsz) in groups_of[j])
        align_pss = [None, None]

        def emit_group(j, gi):
            wq_sb = wq_sbs[j]
            t0, gsz = groups_of[j][gi]
            if align_pss[j] is None:
                align_pss[j] = psA.tile([128, S], F32, tag="align_ps",
                                        name=f"align_ps{j}")
            align_ps = align_pss[j]
            # packed z slices: per t the slice width is its segment's S_eff
            ts = list(range(t0, t0 + gsz))
            widths = [seff(j, t // SEG) for t in ts]
            offs = np.cumsum([0] + [2 * w for w in widths]).tolist()
            z = pz.tile([128, offs[-1]], BF16, tag="z",
                        padded_shape=[128, max_fd], name=f"z{j}_{gi}")
            for i, t in enumerate(ts):
                w = widths[i]
                uh_sb = uh_sbs[(j, t // SEG)]
                for h in range(2):
                    nc.vector.tensor_scalar_add(
                        z[:, offs[i] + h * w: offs[i] + (h + 1) * w],
                        uh_sb[:, h, :], wq_sb[:, h, t:t + 1])
            th = pth.tile([128, offs[-1]], BF16, tag="th",
                          padded_shape=[128, max_fd], name=f"th{j}_{gi}")
            nc.scalar.activation(th[...], z[...], TANH)
            for i, t in enumerate(ts):
                w = widths[i]
                k, c = t // 32, t % 32
                for h in range(2):
                    nc.tensor.matmul(
                        align_ps[32 * k:32 * (k + 1), :w],
                        vcols[:, h, c, :],
                        th[:, offs[i] + h * w: offs[i] + (h + 1) * w],
                        start=(c == 0 and h == 0),
                        stop=(c == 31 and h == 1),
                        tile_position=(0, 32 * k))

        def emit_epilogue(j):
            align_ps = align_pss[j]

            # additive mask, one rank-1 matmul per segment; the first also
            # SETS any never-written tail columns (has_written semantics)
            for k in range(2):
                nc.tensor.matmul(align_ps[...], indics[k][...],
                                 masksegs[j][k][...],
                                 start=False, stop=(k == 1),
                                 skip_group_check=True)

            # softmax (no max-sub: |align| <= ~10); sum fused into the exp
            av_e = pep.tile([128, S], F32, tag="av_e")
            ssum = pep.tile([128, 1], F32, tag="ssum")
            nc.scalar.activation(av_e[...], align_ps[...], EXP,
                                 accum_out=ssum[...])

            # x-part of the output projection + bout: no softmax dependency;
            # single accumulation group for the bank (start only once)
            at_ps = psO.tile([128, 4 * TT], F32, tag="at_ps")
            for oc in range(4):
                for ic in range(4):
                    nc.tensor.matmul(at_ps[:, oc * TT:(oc + 1) * TT],
                                     woXT[:, ic, oc * 128:(oc + 1) * 128],
                                     xTc[j][ic][...],
                                     start=(oc == 0 and ic == 0), stop=False)
                nc.tensor.matmul(at_ps[:, oc * TT:(oc + 1) * TT],
                                 boutw[:, oc * 128:(oc + 1) * 128],
                                 ones1[...],
                                 start=False, stop=False,
                                 skip_group_check=True)

            rcp = pep.tile([128, 1], F32, tag="rcp")
            nc.vector.reciprocal(rcp[...], ssum[...])
            # bf16 copy first: it gates the transpose -> c chain
            av_bf = pep.tile([128, S], BF16, tag="av_bf")
            nc.vector.tensor_scalar_mul(av_bf[...], av_e[...], rcp[...])

            # transpose av -> [s, t]; one tile per s-block
            avTs = []
            for sb in range(4):
                tp = psT.tile([128, 128], BF16, tag="tp")
                nc.tensor.transpose(tp[...], av_bf[:, sb * 128:(sb + 1) * 128],
                                    ident[...])
                avT = pep.tile([128, TT], BF16, tag=f"avT{sb}")
                nc.vector.tensor_copy(avT[...], tp[...])
                avTs.append(avT)

            # align_vectors output (overlaps the c matmuls)
            av = pep.tile([128, S], F32, tag="av")
            nc.vector.tensor_scalar_mul(av[...], av_e[...], rcp[...])
            nc.sync.dma_start(align_d.ap()[j], av[...])

            # c[t,m] laid out [m_half(128), mh, t]; per-segment column
            # slices (each segment contracts only its own mems chunks)
            c_ps = psWC.tile([128, 2 * TT], F32, tag="wqc_ps")
            first = True
            for mh in range(2):
                for k in range(2):
                    nch = (seff(j, k) + 127) // 128
                    for sb in range(nch):
                        nc.tensor.matmul(
                            c_ps[:, mh * TT + k * SEG: mh * TT + (k + 1) * SEG],
                            memsLs[(j, k)][:, sb, mh * 128:(mh + 1) * 128],
                            avTs[sb][:, k * SEG:(k + 1) * SEG],
                            start=first, stop=False,
                            skip_group_check=True)
                        first = False
                c_bf = pep.tile([128, TT], BF16, tag=f"c_bf{mh}")
                nc.vector.tensor_copy(c_bf[...], c_ps[:, mh * TT:(mh + 1) * TT])
                if mh == 0:
                    c_bf0 = c_bf
                else:
                    c_bf1 = c_bf
            c_bfs = [c_bf0, c_bf1]

            # c-part accumulates onto the x-part; per-oc chains so eviction
            # and output DMA pipeline with the remaining matmuls
            for oc in range(4):
                for mh in range(2):
                    nc.tensor.matmul(at_ps[:, oc * TT:(oc + 1) * TT],
                                     woCT[:, mh, oc * 128:(oc + 1) * 128],
                                     c_bfs[mh][...],
                                     start=False, stop=(oc == 3 and mh == 1))
                attn_sb = pep.tile([128, TT], F32, tag=f"attn_sb{oc}")
                nc.vector.tensor_copy(attn_sb[...],
                                      at_ps[:, oc * TT:(oc + 1) * TT])
                nc.sync.dma_start(attn_d.ap()[j][:, oc, :], attn_sb[...])

        # schedule: slot-0 groups with slot-1 prologue hoisted mid-stream;
        # slot-1's first groups before slot-0's epilogue so the DVE keeps
        # feeding the ACT across the transition
        done_p11, done_u01 = False, False
        for gi in range(len(groups_of[0])):
            t0 = groups_of[0][gi][0]
            if t0 >= 36 and not done_p11:
                wq_sbs[1] = phase1_wq(1)
                uh_sbs[(1, 0)] = phase1_uh(1, 0)
                uh_sbs[(1, 1)] = phase1_uh(1, 1)
                done_p11 = True
            if t0 >= 44 and not done_u01:
                uh_sbs[(0, 1)] = phase1_uh(0, 1)
                done_u01 = True
            emit_group(0, gi)
        emit_group(1, 0)
        emit_group(1, 1)
        emit_epilogue(0)
        for gi in range(2, len(groups_of[1])):
            emit_group(1, gi)
        emit_epilogue(1)

    nc.compile()
    return nc


def _to_chunks(a, nch):
    """[nch*128, F] -> [nch, 128, F] (partition-chunked SBUF layout)."""
    return np.ascontiguousarray(a.reshape(nch, 128, a.shape[-1]))


def _to_pcf(a, nch):
    """[nch*128, F] -> [128, nch, F] (single-tile chunked free layout)."""
    return np.ascontiguousarray(a.reshape(nch, 128, a.shape[-1]).transpose(1, 0, 2))


def _prep_inputs(inputs, mems, mem_masks, Wq, Wc, bc, v, Wout, bout):
    x = np.ascontiguousarray(np.asarray(inputs, dtype=np.float32))
    mems = np.ascontiguousarray(np.asarray(mems, dtype=np.float32))
    L = np.asarray(mem_masks).astype(np.int64)
    Wq = np.asarray(Wq, dtype=np.float32)
    Wc = np.asarray(Wc, dtype=np.float32)
    bc = np.asarray(bc, dtype=np.float32)
    v = np.asarray(v, dtype=np.float32)
    Wout = np.asarray(Wout, dtype=np.float32)
    bout = np.asarray(bout, dtype=np.float32)

    seffs = tuple(int(min(max(((int(l) + 1) // 2) * 2, 2), S)) for l in L)

    WqT = _to_chunks(np.ascontiguousarray(Wq.T), 4).astype(BF)
    WcT = _to_chunks(np.ascontiguousarray(Wc.T), 2).astype(BF)
    WoCT = _to_pcf(np.ascontiguousarray(Wout[:, :D].T), 2).astype(BF)
    WoXT = _to_pcf(np.ascontiguousarray(Wout[:, D:].T), 4).astype(BF)
    ident = np.eye(128, dtype=np.float32).astype(BF)
    bc2 = np.ascontiguousarray(bc.reshape(2, 128).T).astype(np.float32)
    vcols = np.zeros((128, 2, 32, 32), np.float32)
    for h in range(2):
        for c in range(32):
            vcols[:, h, c, c] = v[h * 128:(h + 1) * 128]
    vcols = vcols.astype(BF)
    indic = np.zeros((2, 1, 128), np.float32)
    indic[0, 0, :SEG] = 1.0
    indic[1, 0, SEG:] = 1.0

    shared = dict(WqT=WqT, WcT=WcT, vcols=vcols, WoCT=WoCT, WoXT=WoXT,
                  ident=ident, bc2=bc2,
                  indic=indic.astype(BF),
                  boutw=bout.reshape(1, IN).astype(BF),
                  ones1=np.ones((1, 128), np.float32).astype(BF))

    in_maps = []
    for core in range(NC):
        r0 = core * SEG
        xT = np.zeros((NJ, 4, 128, TT), np.float32)
        memsT = np.zeros((NJ, 2, 2, 128, S), np.float32)
        memsL = np.zeros((NJ, 2, 128, 4, D), np.float32)
        maskseg = np.zeros((NJ, 2, 1, S), np.float32)
        for j in range(NJ):
            xrows = np.concatenate(
                [x[2 * j, r0:r0 + SEG, :], x[2 * j + 1, r0:r0 + SEG, :]], 0)
            xT[j] = _to_chunks(np.ascontiguousarray(xrows.T), 4)
            for k in range(2):
                b = 2 * j + k
                memsT[j, k] = _to_chunks(np.ascontiguousarray(mems[b].T), 2)
                memsL[j, k] = _to_pcf(mems[b], 4)
                maskseg[j, k, 0, :] = np.where(np.arange(S) < int(L[b]),
                                               0.0, -30.0)
        m = dict(shared)
        m["xT"] = np.ascontiguousarray(xT).astype(BF)
        m["memsT"] = np.ascontiguousarray(memsT).astype(BF)
        m["memsL"] = np.ascontiguousarray(memsL).astype(BF)
        m["maskseg"] = np.ascontiguousarray(maskseg).astype(BF)
        in_maps.append(m)
    return in_maps, seffs


def kernel(**inputs):
    global LAST_RESULT
    in_maps, seffs = _prep_inputs(**inputs)
    if seffs not in _BUILT:
        _BUILT[seffs] = _build(seffs)
    res = run_bass_kernel_spmd(_BUILT[seffs], in_maps, core_ids=list(range(NC)))
    LAST_RESULT = res

    attn_h = np.zeros((B, T, IN), np.float32)
    align_v = np.zeros((B, T, S), np.float32)
    for core in range(NC):
        r0 = core * SEG
        for j in range(NJ):
            at = res.results[core]["attn_outT"][j]        # [128(p), 4(oc), 128(t)]
            blk = np.transpose(at, (2, 1, 0)).reshape(TT, IN)
            al = res.results[core]["align_out"][j]        # [128(t), 512]
            for k in range(2):
                b = 2 * j + k
                attn_h[b, r0:r0 + SEG, :] = blk[k * SEG:(k + 1) * SEG]
                align_v[b, r0:r0 + SEG, :] = al[k * SEG:(k + 1) * SEG]
    return attn_h, align_v

